# revision 7
# baseline (speedup 1.0000x reference)
"""Trainium2 Bass kernel for nn_Equivariant_SecondOrder_Decoder.

Data-parallel over batch B=8 across 8 NeuronCores (one sample per core).

Per-core math (M=384, D=H=64, S=8):
  x1 = relu(L1(P))        L1 output is broadcast-structured (row+col+diag+const)
  x2 = relu(L2(x1))       L2/L3: 15-basis equivariant layer_2_to_2
  x3 = relu(L3(x2))
  y  = W2^T relu(W1^T x3 + b1) + b2   (channel MLP, fused into L3 consumer)

On-chip layout ("checkerboard", ckb): element (d, i, j) of a [64, 384, 384]
tensor lives at SBUF partition p = d + 64*((i+j)&1), free offset
(m = i>>1, jj = 192*(i&1) + (j>>1)) of a [128, 192, 384] bf16 tile. Every
matmul needed by layer_2_to_2 then lands on PE quadrants (0,0)/(64,64) only
(the (row 64, col 0) quadrant faults on this HW), with contiguous psum writes.

Per pair (rows 2m, 2m+1), one psum bank ps[128, 384]:
  cols [0:192]   = row 2m    (lower part. = even j, upper = odd j)
  cols [192:384] = row 2m+1  (lower = odd j, upper = even j)
mm sequence: identity-opener (injects col-broadcast vector, start=True),
2x A-term (K=128 block-diag W9), 4x C-term (K=64 W10, strided column rhs).
Evac: relu(ps + rowconst-bias) -> bf16 (+ accum_out row sums). Column sums
accumulate via bf16 adds. Diagonals are fixed by small analytic side passes
(the full-term diagonal equals (W9+W10)^T diag_in).

X1 is SBUF resident; X2 streams to DRAM during L2 and is read back into the
same buffer for L3; X3 is consumed by the MLP pair-by-pair (never stored).
The MLP moves odd-parity channel data to partitions 0-63 via SBUF->SBUF DMA
so all its matmuls are quadrant-legal. y's diagonal is patched on the host
from a small device-exported vector.
"""
import numpy as np
import ml_dtypes
import concourse.bacc as bacc
import concourse.mybir as mybir
from concourse import tile
from concourse.bass_utils import run_bass_kernel_spmd

F32 = mybir.dt.float32
BF16 = mybir.dt.bfloat16
AF = mybir.ActivationFunctionType
OP = mybir.AluOpType

B, D, H, M, S = 8, 64, 64, 384, 8
NP = M // 2          # 192 row pairs
HB = M // 2          # half-block width (192)
G = 2 * H            # 128 MLP hidden
CS_GRP = 16          # colsum bf16 accumulation group size

_CACHED_NC = None


def _pg(v):
    """[..., 384] index-ordered -> pg order (evens then odds)."""
    return np.concatenate([v[..., 0::2], v[..., 1::2]], axis=-1)


def _colcmb(col):
    """col [64, 384] (j-indexed) -> opener rhs [128, 384] f32."""
    out = np.empty((128, M), np.float32)
    out[0:64, 0:HB] = col[:, 0::2]
    out[0:64, HB:M] = col[:, 1::2]
    out[64:128, 0:HB] = col[:, 1::2]
    out[64:128, HB:M] = col[:, 0::2]
    return out


def _dup(v):
    return np.concatenate([v, v], axis=0)


# ---------------------------------------------------------------------------
# device program
# ---------------------------------------------------------------------------

def _build_nc():
    nc = bacc.Bacc(None, target_bir_lowering=False)

    def din(name, shape, dt=F32):
        return nc.dram_tensor(name, shape, dt, kind="ExternalInput")

    rc1_d = din("rc1", [128, M])
    cc1_d = din("cc1", [128, M])
    dgfx1_d = din("dgfx1", [64, M])           # relu(diagfix1), pg-i
    dgdel1_d = din("dgdel1", [64, M])         # stat correction, pg-i
    layer_ins = []
    for L in (2, 3):
        ins = {"w9blk": din(f"w9blk{L}", [128, 128], BF16),
               "w10": din(f"w10{L}", [128, H], BF16),
               "wdiag": din(f"wdiag{L}", [64, H], BF16)}
        for nm in ("wc_sc", "wc_sr", "wr_sc", "wr_sr", "wd_sr", "wd_sc",
                   "wsc_sd", "wsc_sa", "wdv_sd", "wdv_sa"):
            ins[nm] = din(f"{nm}{L}", [64, H])
        for nm in ("wc_dg", "wr_dg", "wd_dg"):
            ins[nm] = din(f"{nm}{L}", [64, H], BF16)
        ins["biases"] = din(f"biases{L}", [64, 2])
        layer_ins.append(ins)
    ident_d = din("ident", [128, 128])
    w1_d = din("w1", [64, G], BF16)
    w2_d = din("w2", [G, S], BF16)
    b1_d = din("b1", [128, 1])
    b2p_d = din("b2p", [128, 1])

    x2_d = nc.dram_tensor("x2d", [NP, 128, M], BF16)
    y_d = nc.dram_tensor("y", [NP // 2, 4, S, M], F32, kind="ExternalOutput")
    dgfx3_d = nc.dram_tensor("dgfx3", [64, M], F32, kind="ExternalOutput")

    with tile.TileContext(nc) as tc:
        with (
            tc.tile_pool(name="big", bufs=1) as bigp,
            tc.tile_pool(name="const", bufs=1) as cp,
            tc.tile_pool(name="stat", bufs=1) as sp,
            tc.tile_pool(name="work", bufs=4) as wp,
        ):
            X = bigp.tile([128, NP, M], BF16)
            ident = cp.tile([128, 128], F32)
            nc.sync.dma_start(ident[:], ident_d[:])
            w1 = cp.tile([64, G], BF16)
            nc.sync.dma_start(w1[:], w1_d[:])
            w2 = cp.tile([G, S], BF16)
            nc.sync.dma_start(w2[:], w2_d[:])
            b1 = cp.tile([128, 1], F32)
            nc.sync.dma_start(b1[:], b1_d[:])
            b2p = cp.tile([128, 1], F32)
            nc.sync.dma_start(b2p[:], b2p_d[:])

            rc = cp.tile([128, M], F32, tag="rc")
            cc = cp.tile([128, M], F32, tag="cc")
            w9blk = cp.tile([128, 128], BF16, tag="w9blk")
            w10 = cp.tile([128, H], BF16, tag="w10")
            rs = sp.tile([128, M], F32, tag="rs")
            csacc = sp.tile([128, M], F32, tag="csacc")
            csg = sp.tile([128, M], BF16, tag="csg")
            zero128 = cp.tile([128, M], F32, tag="zero")
            nc.gpsimd.memset(zero128[:], 0.0)
            b2cols = cp.tile([128, M], F32, tag="b2cols")
            nc.vector.tensor_scalar_add(b2cols[:], zero128[:], b2p[:])

            def evac(ps, m, dst, use_act, want_accum=True):
                """psum pair -> dst[:, m, :] / dst[:, :] rows, relu+bias+rowsum."""
                for c0, rpos in ((0, m), (HB, HB + m)):
                    ob = (dst[:, m, c0:c0 + HB] if dst is X
                          else dst[:, c0:c0 + HB])
                    acc = rs[:, rpos:rpos + 1] if want_accum else None
                    if use_act:
                        nc.scalar.activation(ob, ps[:, c0:c0 + HB], AF.Relu,
                                             bias=rc[:, rpos:rpos + 1],
                                             accum_out=acc)
                    else:
                        nc.vector.scalar_tensor_tensor(
                            ob, ps[:, c0:c0 + HB], rc[:, rpos:rpos + 1],
                            zero128[:, 0:HB], OP.add, OP.max, accum_out=acc)

            def colsum_add(src_ap, m):
                if m % CS_GRP == 0:
                    nc.vector.tensor_copy(csg[:], src_ap)
                else:
                    nc.vector.tensor_tensor(csg[:], csg[:], src_ap, OP.add)
                if m % CS_GRP == CS_GRP - 1:
                    if m == CS_GRP - 1:
                        nc.vector.tensor_copy(csacc[:], csg[:])
                    else:
                        nc.vector.tensor_tensor(csacc[:], csacc[:], csg[:],
                                                OP.add)

            Xf = X[:].rearrange("p a b -> p (a b)")

            def fix_diag_and_stats(dg_relu_ap, delta_ap, sr_out, sc_out, dg_out):
                """Overwrite X diag with dg_relu (bf16); fold stats + delta."""
                st = M + 1
                nc.vector.tensor_copy(
                    Xf[0:64, 0:(NP - 1) * st + 1:st], dg_relu_ap[:, 0:HB])
                nc.vector.tensor_copy(
                    Xf[0:64, HB:HB + (NP - 1) * st + 1:st], dg_relu_ap[:, HB:M])
                tmp = sp.tile([64, M], F32, tag="t0")
                # sum_rows: same-column partition fold + delta
                nc.sync.dma_start(tmp[:], rs[64:128, :])
                nc.vector.tensor_tensor(sr_out[:], rs[0:64, :], tmp[:], OP.add)
                nc.vector.tensor_tensor(sr_out[:], sr_out[:], delta_ap, OP.add)
                # sum_cols: cross-block partition fold + delta
                nc.sync.dma_start(tmp[:], csacc[64:128, :])
                nc.vector.tensor_tensor(sc_out[:, 0:HB], csacc[0:64, 0:HB],
                                        tmp[:, HB:M], OP.add)
                nc.vector.tensor_tensor(sc_out[:, HB:M], csacc[0:64, HB:M],
                                        tmp[:, 0:HB], OP.add)
                nc.vector.tensor_tensor(sc_out[:], sc_out[:], delta_ap, OP.add)
                nc.vector.tensor_copy(dg_out[:], dg_relu_ap)

            def boundary(ins, sr, sc, dg, dgfx_out, dvtot_out, pb):
                nc.sync.dma_start(w9blk[:], ins["w9blk"][:])
                nc.sync.dma_start(w10[:], ins["w10"][:])
                cw = {}
                for nm in ("wc_sc", "wc_sr", "wr_sc", "wr_sr", "wd_sr",
                           "wd_sc", "wsc_sd", "wsc_sa", "wdv_sd", "wdv_sa"):
                    cw[nm] = sp.tile([64, H], F32, tag="cw_" + nm, name="cw_" + nm)
                    nc.sync.dma_start(cw[nm][:], ins[nm][:])
                for nm in ("wc_dg", "wr_dg", "wd_dg"):
                    cw[nm] = sp.tile([64, H], BF16, tag="cw_" + nm, name="cw_" + nm)
                    nc.sync.dma_start(cw[nm][:], ins[nm][:])
                wdiag = sp.tile([64, H], BF16, tag="wdiag")
                nc.sync.dma_start(wdiag[:], ins["wdiag"][:])
                bia = sp.tile([64, 2], F32, tag="bia")
                nc.sync.dma_start(bia[:], ins["biases"][:])

                sdv = sp.tile([64, 1], F32, tag="sdv")
                dgf32 = sp.tile([64, M], F32, tag="dgf32")
                nc.vector.tensor_copy(dgf32[:], dg[:])
                nc.vector.tensor_reduce(sdv[:], dgf32[:],
                                        mybir.AxisListType.X, OP.add)
                sav = sp.tile([64, 1], F32, tag="sav")
                nc.vector.tensor_reduce(sav[:], sr[:],
                                        mybir.AxisListType.X, OP.add)

                psc = pb.tile([128, M], F32, tag="bnd")
                nc.tensor.matmul(psc[0:64, :], cw["wc_sc"][:], sc[:],
                                 start=True, stop=False)
                nc.tensor.matmul(psc[0:64, :], cw["wc_sr"][:], sr[:],
                                 start=False, stop=False)
                nc.tensor.matmul(psc[0:64, :], cw["wc_dg"][:], dg[:],
                                 start=False, stop=True)
                colvec = sp.tile([64, M], F32, tag="colvec")
                nc.vector.tensor_copy(colvec[:], psc[0:64, :])
                nc.vector.tensor_copy(cc[0:64, :], colvec[:])
                nc.sync.dma_start(cc[64:128, 0:HB], colvec[:, HB:M])
                nc.sync.dma_start(cc[64:128, HB:M], colvec[:, 0:HB])

                psr = pb.tile([128, M], F32, tag="bnd")
                nc.tensor.matmul(psr[0:64, :], cw["wr_sc"][:], sc[:],
                                 start=True, stop=False)
                nc.tensor.matmul(psr[0:64, :], cw["wr_sr"][:], sr[:],
                                 start=False, stop=False)
                nc.tensor.matmul(psr[0:64, :], cw["wr_dg"][:], dg[:],
                                 start=False, stop=True)
                psk = pb.tile([64, 4], F32, tag="bndk")
                nc.tensor.matmul(psk[:, 0:1], cw["wsc_sd"][:], sdv[:],
                                 start=True, stop=False)
                nc.tensor.matmul(psk[:, 0:1], cw["wsc_sa"][:], sav[:],
                                 start=False, stop=True)
                nc.tensor.matmul(psk[:, 1:2], cw["wdv_sd"][:], sdv[:],
                                 start=True, stop=False)
                nc.tensor.matmul(psk[:, 1:2], cw["wdv_sa"][:], sav[:],
                                 start=False, stop=True)
                kvec = sp.tile([64, 2], F32, tag="kvec")
                nc.vector.tensor_copy(kvec[:], psk[:, 0:2])
                constv = sp.tile([64, 1], F32, tag="constv")
                nc.vector.tensor_tensor(constv[:], kvec[:, 0:1], bia[:, 0:1],
                                        OP.add)
                rowt = sp.tile([64, M], F32, tag="rowt")
                nc.vector.tensor_scalar_add(rowt[:], psr[0:64, :], constv[:])
                nc.vector.tensor_copy(rc[0:64, :], rowt[:])
                nc.sync.dma_start(rc[64:128, :], rowt[:])

                psd = pb.tile([128, M], F32, tag="bnd")
                nc.tensor.matmul(psd[0:64, :], cw["wd_sr"][:], sr[:],
                                 start=True, stop=False)
                nc.tensor.matmul(psd[0:64, :], cw["wd_sc"][:], sc[:],
                                 start=False, stop=False)
                nc.tensor.matmul(psd[0:64, :], cw["wd_dg"][:], dg[:],
                                 start=False, stop=True)
                dvs = sp.tile([64, 1], F32, tag="dvs")
                nc.vector.tensor_tensor(dvs[:], kvec[:, 1:2], bia[:, 1:2],
                                        OP.add)
                nc.vector.tensor_scalar_add(dvtot_out[:], psd[0:64, :], dvs[:])
                psf = pb.tile([128, M], F32, tag="bnd")
                nc.tensor.matmul(psf[0:64, :], wdiag[:], dg[:],
                                 start=True, stop=True)
                nc.vector.tensor_tensor(dgfx_out[:], psf[0:64, :], colvec[:],
                                        OP.add)
                nc.vector.tensor_tensor(dgfx_out[:], dgfx_out[:], rowt[:],
                                        OP.add)
                nc.vector.tensor_tensor(dgfx_out[:], dgfx_out[:], dvtot_out[:],
                                        OP.add)

            def pair_mms(src, m, ps):
                """opener + A + C matmuls for out pair m into ps [128, 384]."""
                nc.tensor.matmul(ps[:, :], ident[:], cc[:],
                                 start=True, stop=False,
                                 skip_group_check=True)
                nc.tensor.matmul(ps[:, 0:HB], w9blk[:], src[:, m, 0:HB],
                                 start=False, stop=False,
                                 skip_group_check=True)
                nc.tensor.matmul(ps[:, HB:M], w9blk[:], src[:, m, HB:M],
                                 start=False, stop=False,
                                 skip_group_check=True)
                nc.tensor.matmul(ps[0:64, 0:HB], w10[0:64, :],
                                 src[0:64, :, m], start=False, stop=False,
                                 skip_group_check=True)
                nc.tensor.matmul(ps[64:128, 0:HB], w10[64:128, :],
                                 src[64:128, :, HB + m],
                                 start=False, stop=False, skip_group_check=True)
                nc.tensor.matmul(ps[0:64, HB:M], w10[0:64, :],
                                 src[0:64, :, HB + m], start=False, stop=True,
                                 skip_group_check=True)
                nc.tensor.matmul(ps[64:128, HB:M], w10[64:128, :],
                                 src[64:128, :, m],
                                 start=False, stop=True, skip_group_check=True)

            dgfx = sp.tile([64, M], F32, tag="dgfx")
            dvtot = sp.tile([64, M], F32, tag="dvtot")
            sr1 = sp.tile([64, M], F32, tag="sr")
            sc1 = sp.tile([64, M], F32, tag="sc")
            dg1 = sp.tile([64, M], BF16, tag="dgb")

            # ============ phase 1: L1 + boundary2 + L2 + boundary3 ============
            with (
                tc.tile_pool(name="ps12", bufs=4, space="PSUM") as pp,
                tc.tile_pool(name="psb", bufs=2, space="PSUM") as pb,
            ):
                nc.sync.dma_start(rc[:], rc1_d[:])
                nc.sync.dma_start(cc[:], cc1_d[:])
                dgfx1 = sp.tile([64, M], F32, tag="dgf")
                nc.sync.dma_start(dgfx1[:], dgfx1_d[:])
                dgdel1 = sp.tile([64, M], F32, tag="dgd")
                nc.sync.dma_start(dgdel1[:], dgdel1_d[:])

                for m in range(NP):
                    ps = pp.tile([128, M], F32)
                    nc.tensor.matmul(ps[:, :], ident[:], cc[:],
                                     start=True, stop=True,
                                     skip_group_check=True)
                    evac(ps, m, X, use_act=(m % 2 == 0))
                    colsum_add(X[:, m, :], m)

                dgr1b = sp.tile([64, M], BF16, tag="dgrb")
                nc.vector.tensor_copy(dgr1b[:], dgfx1[:])
                fix_diag_and_stats(dgr1b[:], dgdel1[:], sr1, sc1, dg1)
                boundary(layer_ins[0], sr1, sc1, dg1, dgfx, dvtot, pb)

                for m in range(NP):
                    ps = pp.tile([128, M], F32)
                    pair_mms(X, m, ps)
                    x2p = wp.tile([128, M], BF16, tag="x2p")
                    evac(ps, m, x2p, use_act=(m % 2 == 0))
                    colsum_add(x2p[:], m)
                    nc.sync.dma_start(x2_d[m, :, :], x2p[:])

                # L2 diag side-pass: write value + stat delta
                dgr2b = sp.tile([64, M], BF16, tag="dgrb")
                t_a = sp.tile([64, M], F32, tag="t1")
                nc.vector.tensor_scalar_max(t_a[:], dgfx[:], 0.0)
                nc.vector.tensor_copy(dgr2b[:], t_a[:])
                t_b = sp.tile([64, M], F32, tag="t2")
                nc.vector.tensor_tensor(t_b[:], dgfx[:], dvtot[:], OP.subtract)
                nc.vector.tensor_scalar_max(t_b[:], t_b[:], 0.0)
                delta2 = sp.tile([64, M], F32, tag="delta")
                nc.vector.tensor_tensor(delta2[:], t_a[:], t_b[:], OP.subtract)

                # read X2 back into X (Tile orders this after all L2 reads)
                for m in range(NP):
                    nc.sync.dma_start(X[:, m, :], x2_d[m, :, :])

                sr2 = sp.tile([64, M], F32, tag="sr")
                sc2 = sp.tile([64, M], F32, tag="sc")
                dg2 = sp.tile([64, M], BF16, tag="dgb")
                fix_diag_and_stats(dgr2b[:], delta2[:], sr2, sc2, dg2)
                boundary(layer_ins[1], sr2, sc2, dg2, dgfx, dvtot, pb)
                nc.sync.dma_start(dgfx3_d[:], dgfx[:])

            # ============ phase 2: L3 + fused MLP =============================
            with (
                tc.tile_pool(name="mlp", bufs=3) as mp,
                tc.tile_pool(name="ps3", bufs=3, space="PSUM") as pp3,
                tc.tile_pool(name="psh", bufs=3, space="PSUM") as ph,
                tc.tile_pool(name="psy", bufs=2, space="PSUM") as py_,
            ):
                ybank = [None]
                for m in range(NP):
                    ps = pp3.tile([128, M], F32, tag="ps")
                    pair_mms(X, m, ps)
                    x3 = mp.tile([128, M], BF16, tag="x3")
                    evac(ps, m, x3, use_act=(m % 2 == 0), want_accum=False)
                    scr = mp.tile([64, M], BF16, tag="scr")
                    nc.sync.dma_start(scr[:], x3[64:128, :])
                    pse = ph.tile([128, M], F32, tag="h")
                    nc.tensor.matmul(pse[:, :], w1[:], x3[0:64, :],
                                     start=True, stop=True,
                                     skip_group_check=True)
                    pso = ph.tile([128, M], F32, tag="h")
                    nc.tensor.matmul(pso[:, :], w1[:], scr[:],
                                     start=True, stop=True,
                                     skip_group_check=True)
                    he = mp.tile([128, M], BF16, tag="he_sb")
                    ho = mp.tile([128, M], BF16, tag="ho_sb")
                    if m % 2 == 0:
                        nc.scalar.activation(he[:], pse[:], AF.Relu, bias=b1[:])
                        nc.vector.scalar_tensor_tensor(
                            ho[:], pso[:], b1[:], zero128[:], OP.add, OP.max)
                    else:
                        nc.vector.scalar_tensor_tensor(
                            he[:], pse[:], b1[:], zero128[:], OP.add, OP.max)
                        nc.scalar.activation(ho[:], pso[:], AF.Relu, bias=b1[:])
                    k = m % 2
                    if k == 0:
                        ybank[0] = py_.tile([128, M], F32, tag="y", name="psy")
                        nc.tensor.matmul(ybank[0][:, :], ident[:], b2cols[:],
                                         start=True, stop=False,
                                         skip_group_check=True)
                    psy = ybank[0]
                    nc.tensor.matmul(psy[64 * k:64 * k + 8, :], w2[:], he[:],
                                     start=False, stop=False,
                                     skip_group_check=True,
                                     tile_position=(0, 64 * k))
                    nc.tensor.matmul(psy[64 * k + 32:64 * k + 40, :], w2[:],
                                     ho[:], start=False, stop=(k == 1),
                                     skip_group_check=True,
                                     tile_position=(0, 64 * k + 32))
                    if k == 1:
                        ysb = mp.tile([128, M], F32, tag="ysb")
                        nc.vector.tensor_copy(ysb[:], psy[:])
                        for slab in range(4):
                            nc.sync.dma_start(y_d[m // 2, slab, :, :],
                                              ysb[32 * slab:32 * slab + 8, :])

    nc.compile()
    return nc


# ---------------------------------------------------------------------------
# host side
# ---------------------------------------------------------------------------

def _vec64(x):
    x = np.asarray(x, np.float32).reshape(-1)
    return x if x.size == 64 else np.full(64, float(x[0]), np.float32)


def _prep_layer_coeffs(W, diag_bias, all_bias):
    W = np.asarray(W, np.float32)
    bf = lambda a: np.ascontiguousarray(a).astype(ml_dtypes.bfloat16)
    f32 = lambda a: np.ascontiguousarray(a).astype(np.float32)
    w9, w10 = W[:, :, 9], W[:, :, 10]
    w9blk = np.zeros((128, 128), np.float32)
    w9blk[0:64, 0:64] = w9
    w9blk[64:128, 64:128] = w9
    return {
        "w9blk": bf(w9blk),
        "w10": bf(_dup(w10)),
        "wdiag": bf(w9 + w10),
        "wc_sc": f32(W[:, :, 7] / M), "wc_sr": f32(W[:, :, 8] / M),
        "wc_dg": bf(W[:, :, 12]),
        "wr_sc": f32(W[:, :, 5] / M), "wr_sr": f32(W[:, :, 6] / M),
        "wr_dg": bf(W[:, :, 11]),
        "wd_sr": f32(W[:, :, 2] / M), "wd_sc": f32(W[:, :, 3] / M),
        "wd_dg": bf(W[:, :, 0]),
        "wsc_sd": f32(W[:, :, 13] / M), "wsc_sa": f32(W[:, :, 14] / (M * M)),
        "wdv_sd": f32(W[:, :, 1] / M), "wdv_sa": f32(W[:, :, 4] / (M * M)),
        "biases": f32(np.stack([_vec64(all_bias), _vec64(diag_bias)], axis=1)),
    }


def _host_prep(P, coeffs0, bias0, l2c, l3c, mlp_w1, mlp_b1, mlp_w2, mlp_b2):
    P = np.asarray(P, np.float32)
    c0 = np.asarray(coeffs0, np.float32)
    bias0v = _vec64(bias0)
    s = P.sum(-1) / M
    proj = lambda w, v: np.einsum('dh,bd...->bh...', w, v)
    row = proj(c0[:, :, 2], P)
    col = proj(c0[:, :, 3], P)
    dvec = proj(c0[:, :, 0], P) + proj(c0[:, :, 1], s)[..., None]
    const = proj(c0[:, :, 4], s)
    rcb = row + const[..., None] + bias0v[None, :, None]
    dgfx = rcb + col + dvec
    dgfx_nod = rcb + col
    delta1 = np.maximum(dgfx, 0) - np.maximum(dgfx_nod, 0)

    shared = {}
    for k, v in _prep_layer_coeffs(*l2c).items():
        shared[k + "2"] = v
    for k, v in _prep_layer_coeffs(*l3c).items():
        shared[k + "3"] = v
    shared["ident"] = np.eye(128, dtype=np.float32)
    shared["w1"] = np.ascontiguousarray(np.asarray(mlp_w1)).astype(
        ml_dtypes.bfloat16)
    shared["w2"] = np.ascontiguousarray(np.asarray(mlp_w2)).astype(
        ml_dtypes.bfloat16)
    shared["b1"] = np.asarray(mlp_b1, np.float32).reshape(G, 1)
    b2p = np.zeros((128, 1), np.float32)
    b2 = np.asarray(mlp_b2, np.float32).reshape(S)
    for slab in range(4):
        b2p[32 * slab:32 * slab + 8, 0] = b2
    shared["b2p"] = b2p

    in_maps = []
    for b in range(B):
        im = dict(shared)
        im["rc1"] = np.ascontiguousarray(_dup(_pg(rcb[b])))
        im["cc1"] = _colcmb(col[b])
        im["dgfx1"] = np.ascontiguousarray(
            _pg(np.maximum(dgfx[b], 0))).astype(np.float32)
        im["dgdel1"] = np.ascontiguousarray(_pg(delta1[b])).astype(np.float32)
        in_maps.append(im)
    return in_maps


def _assemble_y(yd):
    """y_d [96, 4, S, 384] -> [S, 384, 384]."""
    y = np.empty((S, M, M), np.float32)
    mm = np.arange(NP // 2)
    ce = 2 * np.arange(HB)
    for sl in range(4):
        pair_row = 2 * mm + (sl // 2)
        lo = yd[:, sl, :, 0:HB].transpose(1, 0, 2)   # [S, 96, 192]
        hi = yd[:, sl, :, HB:M].transpose(1, 0, 2)
        r0 = 2 * pair_row
        r1 = r0 + 1
        if sl % 2 == 0:
            y[:, r0[:, None], ce[None, :]] = lo
            y[:, r1[:, None], ce[None, :] + 1] = hi
        else:
            y[:, r0[:, None], ce[None, :] + 1] = lo
            y[:, r1[:, None], ce[None, :]] = hi
    return y


def _patch_y_diag(y, dgfx3_pg, mlp_w1, mlp_b1, mlp_w2, mlp_b2):
    x3d = np.empty((64, M), np.float32)
    x3d[:, 0::2] = dgfx3_pg[:, 0:HB]
    x3d[:, 1::2] = dgfx3_pg[:, HB:M]
    x3d = np.maximum(x3d, 0)
    h = np.maximum(np.einsum('di,dg->gi', x3d, np.asarray(mlp_w1, np.float32))
                   + np.asarray(mlp_b1, np.float32)[:, None], 0)
    yd = np.einsum('gi,gs->si', h, np.asarray(mlp_w2, np.float32)) \
        + np.asarray(mlp_b2, np.float32).reshape(S)[:, None]
    ii = np.arange(M)
    y[:, ii, ii] = yd
    return y


def _get_nc():
    global _CACHED_NC
    if _CACHED_NC is None:
        _CACHED_NC = _build_nc()
    return _CACHED_NC


def kernel(P, mask, coeffs0, bias0, coeffs1, diag_bias1, all_bias1,
           coeffs2, diag_bias2, all_bias2, mlp_w1, mlp_b1, mlp_w2, mlp_b2,
           _trace=False):
    in_maps = _host_prep(
        np.asarray(P), np.asarray(coeffs0), np.asarray(bias0),
        (coeffs1, diag_bias1, all_bias1), (coeffs2, diag_bias2, all_bias2),
        mlp_w1, mlp_b1, mlp_w2, mlp_b2)
    nc = _get_nc()
    res = run_bass_kernel_spmd(nc, in_maps, list(range(B)), trace=_trace)
    ys = []
    for b in range(B):
        y = _assemble_y(res.results[b]["y"])
        y = _patch_y_diag(y, res.results[b]["dgfx3"],
                          mlp_w1, mlp_b1, mlp_w2, mlp_b2)
        ys.append(y)
    out = np.stack(ys, axis=0).astype(np.float32)
    if _trace:
        return out, res
    return out
